# revision 20
# baseline (speedup 1.0000x reference)
"""Trainium2 Bass kernel for a dense pre-norm transformer block.

Reference computation (fp32):
    h = LN1(x); qkv = h @ qkv_w + qkv_b; attention (16 heads, no 1/sqrt(d));
    x = x + attn_out @ proj_w + proj_b;
    h2 = LN2(x); x = x + gelu_exact(h2 @ fc1_w + fc1_b) @ fc2_w + fc2_b

Shapes: x [2, 2048, 1024], heads 16 x 64, MLP 4096.

Sharding (8 NeuronCores, Megatron-style tensor parallel over heads):
    cores 0-3 -> batch 0, cores 4-7 -> batch 1. Within a 4-core group,
    core r owns HEADS 4r..4r+3 for attention (Q/K/V/scores/AV computed for
    those heads over ALL 2048 tokens -> no replicated K/V GEMMs), then a
    head-partial projection produces partial x2 for all tokens, which a
    bf16 ReduceScatter(add) over the group turns into the full proj output
    for the core's OWN 512-token block. LN2 + MLP + residual run on that
    block only. Everything is absolute token order -> one uniform SPMD
    program; per-core differences live in host-sliced weights.

Layout: activations feature-major [C, tokens]; all matmul operands bf16
    (fp32 PSUM accumulation); LN stats / softmax sums via ones-vector
    matmuls; softmax sums ride the attn@V matmul as a 65th V column; exp
    batched over two PSUM banks per instruction to amortize ACT overhead.
"""

import sys

if "/opt/trn_rl_repo" not in sys.path:
    sys.path.insert(0, "/opt/trn_rl_repo")

from contextlib import ExitStack

import numpy as np

import concourse.bass as bass
import concourse.mybir as mybir
import concourse.tile as tile
from concourse import bacc
from concourse.bass_utils import run_bass_kernel_spmd

F32 = mybir.dt.float32
BF16 = mybir.dt.bfloat16
AF = mybir.ActivationFunctionType
ALU = mybir.AluOpType

DIM = 1024
CT = DIM // 128          # 8 feature tiles
NTOK = 2048              # tokens per batch
NQ = 512                 # own token block (proj output / MLP)
H = 16
HC = 4                   # heads per core
D = 64
GW = HC * D              # 256 qkv columns per core
MLP = 4096
FT = MLP // 128          # 32 mlp feature tiles
EPS = 1e-5
N_CORES = 8
GELU_AF = None  # test hook: set to AF.Identity to bypass gelu in CoreSim
REPLICA_GROUPS = [[0, 1, 2, 3], [4, 5, 6, 7]]


def _dma(nc, out, in_):
    nc.sync.dma_start(out=out, in_=in_)


def _col(v):
    return v.rearrange("(p o) -> p o", o=1)


def _row(v):
    return v.rearrange("(o f) -> o f", o=1)


def _ln_stats(nc, sb_stat, mu_ps, musq_ps, n, ntok_norm):
    """From accumulated sum / sum-of-squares psums [1, n] produce
    rstd [1,n] and mean*rstd [1,n] (bf16 sbuf, packed in one tile)."""
    mean = sb_stat.tile([1, n], F32, tag="mean", bufs=1, name="mean")
    w = sb_stat.tile([1, n], F32, tag="w", bufs=1, name="w")
    nc.vector.tensor_scalar_mul(mean[:], mu_ps[:], 1.0 / ntok_norm)
    nc.vector.tensor_scalar_mul(w[:], musq_ps[:], 1.0 / ntok_norm)
    m2 = sb_stat.tile([1, n], F32, tag="m2", bufs=1, name="m2")
    nc.vector.tensor_mul(m2[:], mean[:], mean[:])
    nc.vector.tensor_sub(w[:], w[:], m2[:])
    nc.vector.tensor_scalar_add(w[:], w[:], EPS)
    nc.vector.reciprocal(m2[:], w[:])
    rm = sb_stat.tile([1, 2 * n], BF16, tag="rm", bufs=4, name="rm")
    rstd = rm[:, 0:n]
    mrs = rm[:, n : 2 * n]
    nc.scalar.activation(rstd, m2[:], AF.Sqrt)
    nc.vector.tensor_mul(mrs, mean[:], rstd)
    return rm


def _attn_unit(nc, psS, psAV, psRB, e_pool, au_pool, sb_stat,
               kT, qT, vaug, yT, ones_row, t, qsl):
    """Attention for one (head-pair tile t, 512-query chunk): scores ->
    batched exp -> attn@V (one kt behind) -> softmax normalize into yT."""
    av2 = [psAV.tile([65, NQ], F32, tag="av", name=f"av{h}") for h in range(2)]
    prev_e = None
    for kt in range(16):
        s2 = psS.tile([128, 2 * NQ], F32, tag="s", name="s")
        for hh in range(2):
            hsl = slice(hh * 64, (hh + 1) * 64)
            nc.tensor.matmul(
                s2[:, hh * NQ : (hh + 1) * NQ],
                kT[t][hsl, kt * 128 : (kt + 1) * 128],
                qT[t][hsl, qsl],
                start=True, stop=True)
        e_t = e_pool.tile([128, 2 * NQ], BF16, tag="e", name="e")
        nc.scalar.activation(e_t[:], s2[:], AF.Exp)
        if prev_e is not None:
            for hh in range(2):
                nc.tensor.matmul(
                    av2[hh][:], vaug[kt - 1][:, 2 * t + hh, :],
                    prev_e[:, hh * NQ : (hh + 1) * NQ],
                    start=(kt == 1), stop=False)
        prev_e = e_t
    for hh in range(2):
        nc.tensor.matmul(
            av2[hh][:], vaug[15][:, 2 * t + hh, :],
            prev_e[:, hh * NQ : (hh + 1) * NQ],
            start=False, stop=True)
        au = au_pool.tile([65, NQ], F32, tag="au", name="au")
        nc.vector.tensor_copy(au[:], av2[hh][:])
        rcp = sb_stat.tile([1, NQ], BF16, tag="rcp", bufs=2, name="rcp")
        with nc.allow_low_precision("softmax 1/sum"):
            nc.vector.reciprocal(rcp[:], au[64:65, :])
        rb = psRB.tile([64, NQ], F32, tag="rb", name="rb")
        nc.tensor.matmul(rb[:], ones_row[:, 0:64], rcp[:], start=True, stop=True)
        nc.vector.tensor_mul(yT[t][hh * 64 : (hh + 1) * 64, qsl],
                             au[0:64, :], rb[:])


def _ln_qkv_chunk(nc, psAb, psB, ln_work, gp_row, ln1b_t, qb_q, qb_k, ones8,
                  stats, x_t, h1, wk_t, wq_t, wv_t, kT, qT, vaug, ch):
    """LN1 pass 2 for one 512-token chunk, then K/Q/V of that chunk."""
    csl = slice(ch * NQ, (ch + 1) * NQ)
    rm = stats[ch]
    for ct in range(CT):
        grow = gp_row[0:1, ct * 128 : (ct + 1) * 128]
        bc = psAb.tile([128, 2 * NQ], F32, tag="bc", name="bc")
        nc.tensor.matmul(bc[:, 0:NQ], grow, rm[:, 0:NQ], start=True, stop=True)
        nc.tensor.matmul(bc[:, NQ : 2 * NQ], grow, rm[:, NQ : 2 * NQ],
                         start=True, stop=True)
        t = ln_work.tile([128, NQ], BF16, tag="lnt", name="lnt")
        nc.vector.tensor_mul(t[:], x_t[ct][:, csl], bc[:, 0:NQ])
        nc.vector.scalar_tensor_tensor(
            h1[ct][:, csl], t[:], ln1b_t[:, ct : ct + 1],
            bc[:, NQ : 2 * NQ], op0=ALU.add, op1=ALU.subtract)
    # K/Q for this chunk
    for t in range(2):
        tsl = slice(t * 128, (t + 1) * 128)
        ps = psB.tile([128, NQ], F32, tag="bps", name="bps")
        for ct in range(CT):
            nc.tensor.matmul(ps[:], wk_t[ct][:, tsl], h1[ct][:, csl],
                             start=(ct == 0), stop=(ct == CT - 1))
        nc.vector.tensor_scalar_add(kT[t][:, csl], ps[:], qb_k[:, t : t + 1])
        ps = psB.tile([128, NQ], F32, tag="bps", name="bps")
        for ct in range(CT):
            nc.tensor.matmul(ps[:], wq_t[ct][:, tsl], h1[ct][:, csl],
                             start=(ct == 0), stop=(ct == CT - 1))
        nc.vector.tensor_scalar_add(qT[t][:, csl], ps[:], qb_q[:, t : t + 1])
    # V for this chunk's 4 token-tiles
    for tt in range(4 * ch, 4 * ch + 4):
        tsl = slice(tt * 128, (tt + 1) * 128)
        ps = psB.tile([128, GW], F32, tag="bps", name="vps")
        for ct in range(CT):
            nc.tensor.matmul(ps[:], h1[ct][:, tsl], wv_t[ct][:],
                             start=(ct == 0), stop=(ct == CT - 1))
        nc.vector.tensor_copy(vaug[tt][:, :, 0:64],
                              ps[:].rearrange("p (a f) -> p a f", f=64))
        nc.vector.tensor_copy(vaug[tt][:, :, 64:65],
                              ones8[:, 0:HC].rearrange("p (a o) -> p a o", o=1))


def build_program(reps=1):
    nc = bacc.Bacc("TRN2", target_bir_lowering=False)

    xT = nc.declare_dram_parameter("xT", [DIM, NTOK], BF16, isOutput=False)
    xresT = nc.declare_dram_parameter("xresT", [DIM, NQ], BF16, isOutput=False)
    wqT = nc.declare_dram_parameter("wqT", [DIM, GW], BF16, isOutput=False)
    wkT = nc.declare_dram_parameter("wkT", [DIM, GW], BF16, isOutput=False)
    wvT = nc.declare_dram_parameter("wvT", [DIM, GW], BF16, isOutput=False)
    proj_wT = nc.declare_dram_parameter("proj_wT", [GW, DIM], BF16, isOutput=False)
    fc1_w = nc.declare_dram_parameter("fc1_w", [DIM, MLP], BF16, isOutput=False)
    fc2_w = nc.declare_dram_parameter("fc2_w", [MLP, DIM], BF16, isOutput=False)
    ones_in = nc.declare_dram_parameter("ones_in", [128], BF16, isOutput=False)
    bias_pack = nc.declare_dram_parameter("bias_pack", [128, 96], F32, isOutput=False)
    g_rows = nc.declare_dram_parameter("g_rows", [2 * DIM], BF16, isOutput=False)
    outT = nc.declare_dram_parameter("outT", [DIM, NQ], F32, isOutput=True)

    with tile.TileContext(nc, pool_alloc_mode="queue") as tc:
        for _rep in range(reps):
            _build_rep(nc, tc, xT, xresT, wqT, wkT, wvT, proj_wT, fc1_w, fc2_w,
                       ones_in, bias_pack, g_rows, outT)

    nc.compile()
    return nc


def _build_rep(nc, tc, xT, xresT, wqT, wkT, wvT, proj_wT, fc1_w, fc2_w,
               ones_in, bias_pack, g_rows, outT):
            with ExitStack() as _st:
                const = _st.enter_context(tc.tile_pool(name="const", bufs=1))
                xres_pool = _st.enter_context(tc.tile_pool(name="xres", bufs=CT))
                yT_pool = _st.enter_context(tc.tile_pool(name="yTp", bufs=2))
                sb_stat = _st.enter_context(tc.tile_pool(name="stat", bufs=1))
                dram = _st.enter_context(tc.tile_pool(name="dram", bufs=1, space="DRAM"))
                w1_pool = _st.enter_context(tc.tile_pool(name="w1p", bufs=FT))
                # ---- constants ----
                ones_col = const.tile([128, 1], BF16, tag="ones_col")
                _dma(nc, ones_col[:], _col(ones_in[:]))
                ones_row = const.tile([1, 128], BF16, tag="ones_row")
                _dma(nc, ones_row[:], _row(ones_in[:]))
                # packed per-partition bias columns:
                # ln1g ln1b ln2g ln2b projb fc2b (8 each) fc1b (32)
                # qb_q (2) qb_k (2) ones (12) -> 96 cols
                bp = const.tile([128, 96], F32, tag="bp")
                _dma(nc, bp[:], bias_pack[:, :])
                ln1b_t = bp[:, 8:16]
                ln2b_t = bp[:, 24:32]
                projb_t = bp[:, 32:40]
                fc2b_t = bp[:, 40:48]
                fc1b_t = bp[:, 48:80]
                qb_q = bp[:, 80:82]
                qb_k = bp[:, 82:84]
                ones8 = bp[:, 84:92]
                gp_row = const.tile([1, 2 * DIM], BF16, tag="gp_row")
                _dma(nc, gp_row[:], _row(g_rows[:]))

                xres = [xres_pool.tile([128, NQ], BF16, tag="xres", name=f"xres{i}")
                        for i in range(CT)]
                yT = [yT_pool.tile([128, NTOK], BF16, tag="yT", name=f"yT{i}")
                      for i in range(2)]

                # ReduceScatter in two feature-halves so the 2nd overlaps LN2
                rs_in = [dram.tile([4 * DIM // 2, NQ], BF16, name=f"rs_in{i}")
                         for i in range(2)]
                rs_out = [dram.tile([DIM // 2, NQ], BF16, name=f"rs_out{i}")
                          for i in range(2)]

                # fc1 weights prefetched whole (8MB bf16 = 64KB sbuf): DMA
                # streams during LN1/attention; MLP never waits on fc1 loads.
                w1_t = [w1_pool.tile([128, 512], BF16, tag="w1", name=f"w1_{i}")
                        for i in range(FT)]

                with tc.tile_pool(name="h1p", bufs=CT) as h1_pool:
                    h1 = [h1_pool.tile([128, NTOK], BF16, tag="h1", name=f"h1_{i}")
                          for i in range(CT)]

                    # ==== stage A+B: LN1 + Q/K/V, pipelined per 512-tok chunk ====
                    with ExitStack() as _stb:
                        kv_pool = _stb.enter_context(tc.tile_pool(name="kvp", bufs=1))
                        _stx = _stb.enter_context(ExitStack())
                        x_pool1 = _stx.enter_context(tc.tile_pool(name="xp1", bufs=CT))
                        ln_work = _stx.enter_context(tc.tile_pool(name="lnw", bufs=2))
                        w_pool = _stx.enter_context(tc.tile_pool(name="wp", bufs=3 * CT))
                        kT = [kv_pool.tile([128, NTOK], BF16, tag=f"kT{t}", name=f"kT{t}")
                              for t in range(2)]
                        qT = [kv_pool.tile([128, NTOK], BF16, tag=f"qT{t}", name=f"qT{t}")
                              for t in range(2)]
                        vaug = [kv_pool.tile([128, HC, 65], BF16, tag=f"va{t}", name=f"va{t}")
                                for t in range(16)]
                        x_t = [x_pool1.tile([128, NTOK], BF16, tag="xs", name=f"xs{i}")
                               for i in range(CT)]
                        wk_t = [w_pool.tile([128, GW], BF16, tag="wqkv", name=f"wk{i}")
                                for i in range(CT)]
                        wq_t = [w_pool.tile([128, GW], BF16, tag="wqkv", name=f"wq{i}")
                                for i in range(CT)]
                        wv_t = [w_pool.tile([128, GW], BF16, tag="wqkv", name=f"wv{i}")
                                for i in range(CT)]
                        with ExitStack() as _sta:
                            sq_pool = _sta.enter_context(tc.tile_pool(name="sqp", bufs=2))
                            psA = _sta.enter_context(tc.tile_pool(name="psA", bufs=4, space="PSUM"))
                            mu_ps = [psA.tile([1, NQ], F32, tag="mu", name=f"mu{i}")
                                     for i in range(4)]
                            musq_ps = [psA.tile([1, NQ], F32, tag="musq", name=f"musq{i}")
                                       for i in range(4)]
                            # chunk-major: chunk 0 stats complete after 1/4 of DMA
                            stats = []
                            for ch in range(4):
                                for ct in range(CT):
                                    _dma(nc, x_t[ct][:, ch * NQ : (ch + 1) * NQ],
                                         xT[ct * 128 : (ct + 1) * 128,
                                            ch * NQ : (ch + 1) * NQ])
                                if ch == 0:
                                    # qkv weights: after chunk-0 x so LN1 starts asap
                                    for ct in range(CT):
                                        rsl = slice(ct * 128, (ct + 1) * 128)
                                        _dma(nc, wk_t[ct][:], wkT[rsl, :])
                                        _dma(nc, wq_t[ct][:], wqT[rsl, :])
                                        _dma(nc, wv_t[ct][:], wvT[rsl, :])
                                for ct in range(CT):
                                    csl = slice(ch * NQ, (ch + 1) * NQ)
                                    sq = sq_pool.tile([128, NQ], BF16, tag="sq", name="sq")
                                    nc.vector.tensor_mul(sq[:], x_t[ct][:, csl],
                                                         x_t[ct][:, csl])
                                    nc.tensor.matmul(
                                        mu_ps[ch][:], ones_col[:], x_t[ct][:, csl],
                                        start=(ct == 0), stop=(ct == CT - 1))
                                    nc.tensor.matmul(
                                        musq_ps[ch][:], ones_col[:], sq[:],
                                        start=(ct == 0), stop=(ct == CT - 1))
                                stats.append(_ln_stats(nc, sb_stat, mu_ps[ch],
                                                       musq_ps[ch], NQ, DIM))
                            # fc1 prefetch (first half): after x loads
                            for i in range(FT):
                                _dma(nc, w1_t[i][:],
                                     fc1_w[(i % CT) * 128 : (i % CT + 1) * 128,
                                           (i // CT) * 512 : (i // CT + 1) * 512])
                        # per chunk: LN1 pass 2, then K/Q/V of that chunk
                        with ExitStack() as _stp:
                            psAb = _stp.enter_context(tc.tile_pool(name="psAb", bufs=3, space="PSUM"))
                            psB = _stp.enter_context(tc.tile_pool(name="psB", bufs=2, space="PSUM"))
                            for ch in range(4):
                                _ln_qkv_chunk(nc, psAb, psB, ln_work, gp_row,
                                              ln1b_t, qb_q, qb_k, ones8, stats,
                                              x_t, h1, wk_t, wq_t, wv_t,
                                              kT, qT, vaug, ch)
                        _stx.close()  # free x/weights sbuf before attention pools

                        # === stage C+D: attention with proj interleaved per qch ===
                        with ExitStack() as _stc:
                            e_pool = _stc.enter_context(tc.tile_pool(name="ep", bufs=4))
                            au_pool = _stc.enter_context(tc.tile_pool(name="aup", bufs=4))
                            pw_pool = _stc.enter_context(tc.tile_pool(name="pwp", bufs=2))
                            d_sb = _stc.enter_context(tc.tile_pool(name="dsb", bufs=4))
                            psS = _stc.enter_context(tc.tile_pool(name="psS", bufs=2, space="PSUM"))
                            psAV = _stc.enter_context(tc.tile_pool(name="psAV", bufs=2, space="PSUM"))
                            psRB = _stc.enter_context(tc.tile_pool(name="psRB", bufs=1, space="PSUM"))
                            psD = _stc.enter_context(tc.tile_pool(name="psD", bufs=1, space="PSUM"))
                            pw_t = [pw_pool.tile([128, DIM], BF16, tag="pw", name=f"pw{i}")
                                    for i in range(2)]
                            for t in range(2):
                                _dma(nc, pw_t[t][:], proj_wT[t * 128 : (t + 1) * 128, :])
                            for qch in range(4):
                                qsl = slice(qch * NQ, (qch + 1) * NQ)
                                for t in range(2):
                                    _attn_unit(nc, psS, psAV, psRB, e_pool,
                                               au_pool, sb_stat, kT, qT, vaug,
                                               yT, ones_row, t, qsl)
                                # head-partial proj for this token block (j = qch)
                                for co in range(CT):
                                    ps = psD.tile([128, NQ], F32, tag="dps", name="dps")
                                    for t in range(2):
                                        nc.tensor.matmul(
                                            ps[:], pw_t[t][:, co * 128 : (co + 1) * 128],
                                            yT[t][:, qsl],
                                            start=(t == 0), stop=(t == 1))
                                    stg = d_sb.tile([128, NQ], BF16, tag="stg", name="stg")
                                    nc.vector.tensor_copy(stg[:], ps[:])
                                    half, cof = co // 4, co % 4
                                    _dma(nc,
                                         rs_in[half][qch * NQ + cof * 128
                                                     : qch * NQ + (cof + 1) * 128, :],
                                         stg[:])
                            for half in range(2):
                                nc.gpsimd.collective_compute(
                                    "ReduceScatter",
                                    mybir.AluOpType.add,
                                    replica_groups=REPLICA_GROUPS,
                                    ins=[rs_in[half].opt()],
                                    outs=[rs_out[half].opt()],
                                )


                # ====== stage E: residual + LN2 on own 512-token block ======
                with tc.tile_pool(name="x2p", bufs=CT) as x2_pool:
                    x2 = [x2_pool.tile([128, NQ], BF16, tag="x2", name=f"x2_{i}")
                          for i in range(CT)]
                    with tc.tile_pool(name="h2p", bufs=CT) as h2_pool:
                        h2 = [h2_pool.tile([128, NQ], BF16, tag="h2", name=f"h2_{i}")
                              for i in range(CT)]
                        with ExitStack() as _ste:
                            rs_pool = _ste.enter_context(tc.tile_pool(name="rsp", bufs=CT))
                            psE = _ste.enter_context(tc.tile_pool(name="psE", bufs=1, space="PSUM"))
                            psEb = _ste.enter_context(tc.tile_pool(name="psEb", bufs=2, space="PSUM"))
                            sq2_pool = _ste.enter_context(tc.tile_pool(name="sq2p", bufs=2))
                            ln_work2 = _ste.enter_context(tc.tile_pool(name="lnw2", bufs=2))
                            for ct in range(CT):
                                _dma(nc, xres[ct][:],
                                     xresT[ct * 128 : (ct + 1) * 128, :])
                            rs_t = [rs_pool.tile([128, NQ], BF16, tag="rs", name=f"rs{i}")
                                    for i in range(CT)]
                            for ct in range(CT):
                                _dma(nc, rs_t[ct][:],
                                     rs_out[ct // 4][(ct % 4) * 128
                                                     : (ct % 4 + 1) * 128, :])
                            mu_ps = psE.tile([1, NQ], F32, tag="mu2", name="mu2")
                            musq_ps = psE.tile([1, NQ], F32, tag="musq2", name="musq2")
                            for co in range(CT):
                                nc.vector.scalar_tensor_tensor(
                                    x2[co][:], rs_t[co][:], projb_t[:, co : co + 1],
                                    xres[co][:], op0=ALU.add, op1=ALU.add)
                                sq = sq2_pool.tile([128, NQ], BF16, tag="sq2", name="sq2")
                                nc.vector.tensor_mul(sq[:], x2[co][:], x2[co][:])
                                nc.tensor.matmul(mu_ps[:], ones_col[:], x2[co][:],
                                                 start=(co == 0), stop=(co == CT - 1))
                                nc.tensor.matmul(musq_ps[:], ones_col[:], sq[:],
                                                 start=(co == 0), stop=(co == CT - 1))
                            rm = _ln_stats(nc, sb_stat, mu_ps, musq_ps, NQ, DIM)
                            for ct in range(CT):
                                grow = gp_row[0:1, DIM + ct * 128 : DIM + (ct + 1) * 128]
                                bc = psEb.tile([128, 2 * NQ], F32, tag="bc2", name="bc2")
                                nc.tensor.matmul(bc[:, 0:NQ], grow,
                                                 rm[:, 0:NQ],
                                                 start=True, stop=True)
                                nc.tensor.matmul(bc[:, NQ : 2 * NQ], grow,
                                                 rm[:, NQ : 2 * NQ],
                                                 start=True, stop=True)
                                t = ln_work2.tile([128, NQ], BF16, tag="lnt2", name="lnt2")
                                nc.vector.tensor_mul(t[:], x2[ct][:], bc[:, 0:NQ])
                                nc.vector.scalar_tensor_tensor(
                                    h2[ct][:], t[:], ln2b_t[:, ct : ct + 1],
                                    bc[:, NQ : 2 * NQ], op0=ALU.add, op1=ALU.subtract)

                        # ============ stage F: MLP ==============================
                        with tc.tile_pool(name="gp", bufs=FT) as g_pool:
                            g_t = [g_pool.tile([128, NQ], BF16, tag="g", name=f"g{i}")
                                   for i in range(FT)]
                            with (
                                tc.tile_pool(name="w1s", bufs=16) as w1s_pool,
                                tc.tile_pool(name="psF1", bufs=8, space="PSUM") as psF1,
                            ):
                                for fog in range(8):
                                    if fog < 4:
                                        wt = [w1_t[fog * CT + ct] for ct in range(CT)]
                                    else:
                                        wt = [w1s_pool.tile([128, 512], BF16, tag="w1s",
                                                            name=f"w1s{i}")
                                              for i in range(CT)]
                                        for ct in range(CT):
                                            _dma(nc, wt[ct][:],
                                                 fc1_w[ct * 128 : (ct + 1) * 128,
                                                       fog * 512 : (fog + 1) * 512])
                                    pss = [psF1.tile([128, NQ], F32, tag="f1ps",
                                                     name=f"f1ps{i}")
                                           for i in range(4)]
                                    for ct in range(CT):
                                        for fo4 in range(4):
                                            nc.tensor.matmul(
                                                pss[fo4][:],
                                                wt[ct][:, fo4 * 128 : (fo4 + 1) * 128],
                                                h2[ct][:],
                                                start=(ct == 0), stop=(ct == CT - 1))
                                    for fo4 in range(4):
                                        fo = fog * 4 + fo4
                                        nc.scalar.activation(
                                            g_t[fo][:], pss[fo4][:],
                                            GELU_AF or AF.Gelu,
                                            bias=fc1b_t[:, fo : fo + 1])
                            # fc2 + residual
                            with ExitStack() as _stf:
                                w2_pool = _stf.enter_context(tc.tile_pool(name="w2p", bufs=16))
                                psF2 = _stf.enter_context(tc.tile_pool(name="psF2", bufs=8, space="PSUM"))
                                out_pool = _stf.enter_context(tc.tile_pool(name="op", bufs=4))
                                for cog in range(2):
                                    pss = [psF2.tile([128, NQ], F32, tag="f2ps",
                                                     name=f"f2ps{i}")
                                           for i in range(4)]
                                    for ko in range(FT):
                                        w2_t = w2_pool.tile([128, 512], BF16, tag="w2")
                                        _dma(nc, w2_t[:],
                                             fc2_w[ko * 128 : (ko + 1) * 128,
                                                   cog * 512 : (cog + 1) * 512])
                                        for co4 in range(4):
                                            nc.tensor.matmul(
                                                pss[co4][:],
                                                w2_t[:, co4 * 128 : (co4 + 1) * 128],
                                                g_t[ko][:],
                                                start=(ko == 0), stop=(ko == FT - 1))
                                    for co4 in range(4):
                                        co = cog * 4 + co4
                                        o_t = out_pool.tile([128, NQ], F32, tag="o")
                                        nc.vector.scalar_tensor_tensor(
                                            o_t[:], pss[co4][:], fc2b_t[:, co : co + 1],
                                            x2[co][:], op0=ALU.add, op1=ALU.add)
                                        _dma(nc, outT[co * 128 : (co + 1) * 128, :], o_t[:])


_CACHED_NC = None


def _get_nc():
    global _CACHED_NC
    if _CACHED_NC is None:
        _CACHED_NC = build_program()
    return _CACHED_NC


def make_in_maps(inputs):
    from ml_dtypes import bfloat16

    ins = {k: np.ascontiguousarray(np.asarray(v), dtype=np.float32)
           for k, v in inputs.items()}
    proj_b_eff = (ins["proj_b"]
                  + ins["qkv_b"][2048:].astype(np.float64)
                  @ ins["proj_w"].astype(np.float64)).astype(np.float32)
    fc1_wb = np.ascontiguousarray(ins["fc1_w"]).astype(bfloat16)
    fc2_wb = np.ascontiguousarray(ins["fc2_w"]).astype(bfloat16)
    g_rows = np.concatenate([ins["ln1_g"], ins["ln2_g"]]).astype(bfloat16)
    in_maps = []
    for core in range(N_CORES):
        b = core // 4
        r = core % 4
        hsl = slice(r * GW, (r + 1) * GW)
        cols = [ins["ln1_g"], ins["ln1_b"], ins["ln2_g"], ins["ln2_b"],
                proj_b_eff, ins["fc2_b"], ins["fc1_b"],
                ins["qkv_b"][hsl], ins["qkv_b"][1024 + r * GW : 1024 + (r + 1) * GW]]
        packed = np.concatenate(
            [c.reshape(-1, 128).T for c in cols] + [np.ones((128, 12), np.float32)],
            axis=1)
        in_maps.append({
            "xT": np.ascontiguousarray(ins["x"][b].T).astype(bfloat16),
            "xresT": np.ascontiguousarray(
                ins["x"][b][r * NQ : (r + 1) * NQ, :].T).astype(bfloat16),
            "wqT": np.ascontiguousarray(ins["qkv_w"][:, hsl]).astype(bfloat16),
            "wkT": np.ascontiguousarray(
                ins["qkv_w"][:, 1024 + r * GW : 1024 + (r + 1) * GW]).astype(bfloat16),
            "wvT": np.ascontiguousarray(
                ins["qkv_w"][:, 2048 + r * GW : 2048 + (r + 1) * GW]).astype(bfloat16),
            "proj_wT": np.ascontiguousarray(ins["proj_w"][hsl, :]).astype(bfloat16),
            "fc1_w": fc1_wb,
            "fc2_w": fc2_wb,
            "ones_in": np.ones(128, bfloat16),
            "bias_pack": np.ascontiguousarray(packed),
            "g_rows": g_rows,
        })
    return in_maps


def gather_output(results):
    out = np.empty((2, NTOK, DIM), dtype=np.float32)
    for core in range(N_CORES):
        b = core // 4
        qs = (core % 4) * NQ
        out[b, qs : qs + NQ, :] = results[core]["outT"].T
    return out


def kernel(**inputs) -> np.ndarray:
    nc = _get_nc()
    in_maps = make_in_maps(inputs)
    res = run_bass_kernel_spmd(nc, in_maps, list(range(N_CORES)))
    return gather_output(res.results)


if __name__ == "__main__":
    rng = np.random.default_rng(0)
    demo = {
        "x": rng.standard_normal((2, NTOK, DIM), dtype=np.float32),
        "ln1_g": np.ones(DIM, np.float32), "ln1_b": np.zeros(DIM, np.float32),
        "qkv_w": (rng.standard_normal((DIM, 3 * DIM)) * 0.02).astype(np.float32),
        "qkv_b": np.zeros(3 * DIM, np.float32),
        "proj_w": (rng.standard_normal((DIM, DIM)) * 0.02).astype(np.float32),
        "proj_b": np.zeros(DIM, np.float32),
        "ln2_g": np.ones(DIM, np.float32), "ln2_b": np.zeros(DIM, np.float32),
        "fc1_w": (rng.standard_normal((DIM, MLP)) * 0.02).astype(np.float32),
        "fc1_b": np.zeros(MLP, np.float32),
        "fc2_w": (rng.standard_normal((MLP, DIM)) * 0.02).astype(np.float32),
        "fc2_b": np.zeros(DIM, np.float32),
    }
    out = kernel(**demo)
    print("out", out.shape, out.dtype, float(np.abs(out).max()))


# revision 25
# speedup vs baseline: 197.2395x; 197.2395x over previous
"""Trainium2 Bass kernel for a dense pre-norm transformer block.

Reference computation (fp32):
    h = LN1(x); qkv = h @ qkv_w + qkv_b; attention (16 heads, no 1/sqrt(d));
    x = x + attn_out @ proj_w + proj_b;
    h2 = LN2(x); x = x + gelu_exact(h2 @ fc1_w + fc1_b) @ fc2_w + fc2_b

Shapes: x [2, 2048, 1024], heads 16 x 64, MLP 4096.

Sharding (8 NeuronCores, Megatron-style tensor parallel over heads):
    cores 0-3 -> batch 0, cores 4-7 -> batch 1. Within a 4-core group,
    core r owns HEADS 4r..4r+3 for attention (Q/K/V/scores/AV computed for
    those heads over ALL 2048 tokens -> no replicated K/V GEMMs), then a
    head-partial projection produces partial x2 for all tokens, which a
    bf16 ReduceScatter(add) over the group turns into the full proj output
    for the core's OWN 512-token block. LN2 + MLP + residual run on that
    block only. Everything is absolute token order -> one uniform SPMD
    program; per-core differences live in host-sliced weights.

Layout: activations feature-major [C, tokens]; all matmul operands bf16
    (fp32 PSUM accumulation); LN stats / softmax sums via ones-vector
    matmuls; softmax sums ride the attn@V matmul as a 65th V column; exp
    batched over two PSUM banks per instruction to amortize ACT overhead.
"""

import sys

if "/opt/trn_rl_repo" not in sys.path:
    sys.path.insert(0, "/opt/trn_rl_repo")

from contextlib import ExitStack

import numpy as np

import concourse.bass as bass
import concourse.mybir as mybir
import concourse.tile as tile
from concourse import bacc
from concourse.bass_utils import run_bass_kernel_spmd

F32 = mybir.dt.float32
BF16 = mybir.dt.bfloat16
FP8 = mybir.dt.float8e4
W8SCALE = 32.0
AF = mybir.ActivationFunctionType
ALU = mybir.AluOpType

DIM = 1024
CT = DIM // 128          # 8 feature tiles
NTOK = 2048              # tokens per batch
NQ = 512                 # own token block (proj output / MLP)
H = 16
HC = 4                   # heads per core
D = 64
GW = HC * D              # 256 qkv columns per core
MLP = 4096
FT = MLP // 128          # 32 mlp feature tiles
EPS = 1e-5
N_CORES = 8
GELU_AF = None  # test hook: set to AF.Identity to bypass gelu in CoreSim
REPLICA_GROUPS = [[0, 1, 2, 3], [4, 5, 6, 7]]


def _dma(nc, out, in_):
    nc.sync.dma_start(out=out, in_=in_)


def _col(v):
    return v.rearrange("(p o) -> p o", o=1)


def _row(v):
    return v.rearrange("(o f) -> o f", o=1)


def _ln_stats(nc, sb_stat, mu_ps, musq_ps, n, ntok_norm):
    """From accumulated sum / sum-of-squares psums [1, n] produce
    rstd [1,n] and mean*rstd [1,n] (bf16 sbuf, packed in one tile)."""
    mean = sb_stat.tile([1, n], F32, tag="mean", bufs=1, name="mean")
    w = sb_stat.tile([1, n], F32, tag="w", bufs=1, name="w")
    nc.vector.tensor_scalar_mul(mean[:], mu_ps[:], 1.0 / ntok_norm)
    nc.vector.tensor_scalar_mul(w[:], musq_ps[:], 1.0 / ntok_norm)
    m2 = sb_stat.tile([1, n], F32, tag="m2", bufs=1, name="m2")
    nc.vector.tensor_mul(m2[:], mean[:], mean[:])
    nc.vector.tensor_sub(w[:], w[:], m2[:])
    nc.vector.tensor_scalar_add(w[:], w[:], EPS)
    nc.vector.reciprocal(m2[:], w[:])
    rm = sb_stat.tile([1, 2 * n], BF16, tag="rm", bufs=4, name="rm")
    rstd = rm[:, 0:n]
    mrs = rm[:, n : 2 * n]
    nc.scalar.activation(rstd, m2[:], AF.Sqrt)
    nc.vector.tensor_mul(mrs, mean[:], rstd)
    return rm


def _proj_co(nc, psD, d_sb, pw_t, yT, rs_in, qch, co):
    """One 128-feature column of the head-partial projection for block qch."""
    ps = psD.tile([128, NQ], F32, tag="dps", name="dps")
    qsl = slice(qch * NQ, (qch + 1) * NQ)
    for t in range(2):
        nc.tensor.matmul(ps[:], pw_t[t][:, co * 128 : (co + 1) * 128],
                         yT[t][:, qsl], start=(t == 0), stop=(t == 1))
    stg = d_sb.tile([128, NQ], BF16, tag="stg", name="stg")
    nc.vector.tensor_copy(stg[:], ps[:])
    half, cof = co // 4, co % 4
    nc.sync.dma_start(
        out=rs_in[half][qch * NQ + cof * 128 : qch * NQ + (cof + 1) * 128, :],
        in_=stg[:])


def _attn_unit(nc, psS, psAV, psRB, e_pool, au_pool, sb_stat,
               kT, qT, vaug, yT, ones_row, t, qsl, side=None):
    """Attention for one (head-pair tile t, 512-query chunk): scores ->
    batched exp -> attn@V (one kt behind) -> softmax normalize into yT."""
    av2 = [psAV.tile([65, NQ], F32, tag="av", name=f"av{h}") for h in range(2)]
    prev_e = None
    for kt in range(16):
        s2 = psS.tile([128, 2 * NQ], F32, tag="s", name="s")
        for hh in range(2):
            hsl = slice(hh * 64, (hh + 1) * 64)
            nc.tensor.matmul(
                s2[:, hh * NQ : (hh + 1) * NQ],
                kT[t][hsl, kt * 128 : (kt + 1) * 128],
                qT[t][hsl, qsl],
                start=True, stop=True)
        e_t = e_pool.tile([128, 2 * NQ], BF16, tag="e", name="e")
        nc.scalar.activation(e_t[:], s2[:], AF.Exp)
        if prev_e is not None:
            for hh in range(2):
                nc.tensor.matmul(
                    av2[hh][:], vaug[kt - 1][:, 2 * t + hh, :],
                    prev_e[:, hh * NQ : (hh + 1) * NQ],
                    start=(kt == 1), stop=False)
        if side is not None and 1 <= kt <= len(side):
            side[kt - 1]()  # interleave prev block's proj between score streams
        prev_e = e_t
    for hh in range(2):
        nc.tensor.matmul(
            av2[hh][:], vaug[15][:, 2 * t + hh, :],
            prev_e[:, hh * NQ : (hh + 1) * NQ],
            start=False, stop=True)
        au = au_pool.tile([65, NQ], F32, tag="au", name="au")
        nc.vector.tensor_copy(au[:], av2[hh][:])
        rcp = sb_stat.tile([1, NQ], BF16, tag="rcp", bufs=2, name="rcp")
        with nc.allow_low_precision("softmax 1/sum"):
            nc.vector.reciprocal(rcp[:], au[64:65, :])
        rb = psRB.tile([64, NQ], F32, tag="rb", name="rb")
        nc.tensor.matmul(rb[:], ones_row[:, 0:64], rcp[:], start=True, stop=True)
        nc.vector.tensor_mul(yT[t][hh * 64 : (hh + 1) * 64, qsl],
                             au[0:64, :], rb[:])


def _ln_qkv_chunk(nc, psAb, psB, ln_work, gp_row, ln1b_t, qb_q, qb_k, ones8,
                  stats, x_t, h1, wk_t, wq_t, wv_t, kT, qT, vaug, ch):
    """LN1 pass 2 for one 512-token chunk, then K/Q/V of that chunk."""
    csl = slice(ch * NQ, (ch + 1) * NQ)
    rm = stats[ch]
    for ct in range(CT):
        grow = gp_row[0:1, ct * 128 : (ct + 1) * 128]
        bc = psAb.tile([128, 2 * NQ], F32, tag="bc", name="bc")
        nc.tensor.matmul(bc[:, 0:NQ], grow, rm[:, 0:NQ], start=True, stop=True)
        nc.tensor.matmul(bc[:, NQ : 2 * NQ], grow, rm[:, NQ : 2 * NQ],
                         start=True, stop=True)
        t = ln_work.tile([128, NQ], BF16, tag="lnt", name="lnt")
        nc.vector.tensor_mul(t[:], x_t[ct][:, csl], bc[:, 0:NQ])
        nc.vector.scalar_tensor_tensor(
            h1[ct][:, csl], t[:], ln1b_t[:, ct : ct + 1],
            bc[:, NQ : 2 * NQ], op0=ALU.add, op1=ALU.subtract)
    # K/Q for this chunk
    for t in range(2):
        tsl = slice(t * 128, (t + 1) * 128)
        ps = psB.tile([128, NQ], F32, tag="bps", name="bps")
        for ct in range(CT):
            nc.tensor.matmul(ps[:], wk_t[ct][:, tsl], h1[ct][:, csl],
                             start=(ct == 0), stop=(ct == CT - 1))
        nc.vector.tensor_scalar_add(kT[t][:, csl], ps[:], qb_k[:, t : t + 1])
        ps = psB.tile([128, NQ], F32, tag="bps", name="bps")
        for ct in range(CT):
            nc.tensor.matmul(ps[:], wq_t[ct][:, tsl], h1[ct][:, csl],
                             start=(ct == 0), stop=(ct == CT - 1))
        nc.vector.tensor_scalar_add(qT[t][:, csl], ps[:], qb_q[:, t : t + 1])
    # V for this chunk's 4 token-tiles
    for tt in range(4 * ch, 4 * ch + 4):
        tsl = slice(tt * 128, (tt + 1) * 128)
        ps = psB.tile([128, GW], F32, tag="bps", name="vps")
        for ct in range(CT):
            nc.tensor.matmul(ps[:], h1[ct][:, tsl], wv_t[ct][:],
                             start=(ct == 0), stop=(ct == CT - 1))
        nc.vector.tensor_copy(vaug[tt][:, :, 0:64],
                              ps[:].rearrange("p (a f) -> p a f", f=64))
        nc.vector.tensor_copy(vaug[tt][:, :, 64:65],
                              ones8[:, 0:HC].rearrange("p (a o) -> p a o", o=1))


def build_program(reps=1):
    nc = bacc.Bacc("TRN2", target_bir_lowering=False)

    xT = nc.declare_dram_parameter("xT", [DIM, NTOK], BF16, isOutput=False)
    xresT = nc.declare_dram_parameter("xresT", [DIM, NQ], BF16, isOutput=False)
    wqT = nc.declare_dram_parameter("wqT", [DIM, GW], BF16, isOutput=False)
    wkT = nc.declare_dram_parameter("wkT", [DIM, GW], BF16, isOutput=False)
    wvT = nc.declare_dram_parameter("wvT", [DIM, GW], BF16, isOutput=False)
    proj_wT = nc.declare_dram_parameter("proj_wT", [GW, DIM], BF16, isOutput=False)
    fc1_w = nc.declare_dram_parameter("fc1_w", [DIM // 2, 2, MLP], FP8, isOutput=False)
    fc2_w = nc.declare_dram_parameter("fc2_w", [MLP, DIM], BF16, isOutput=False)
    ones_in = nc.declare_dram_parameter("ones_in", [128], BF16, isOutput=False)
    bias_pack = nc.declare_dram_parameter("bias_pack", [128, 96], F32, isOutput=False)
    g_rows = nc.declare_dram_parameter("g_rows", [2 * DIM], BF16, isOutput=False)
    outT = nc.declare_dram_parameter("outT", [DIM, NQ], F32, isOutput=True)

    with tile.TileContext(nc, pool_alloc_mode="queue") as tc:
        for _rep in range(reps):
            _build_rep(nc, tc, xT, xresT, wqT, wkT, wvT, proj_wT, fc1_w, fc2_w,
                       ones_in, bias_pack, g_rows, outT)

    nc.compile()
    return nc


def _build_rep(nc, tc, xT, xresT, wqT, wkT, wvT, proj_wT, fc1_w, fc2_w,
               ones_in, bias_pack, g_rows, outT):
            with ExitStack() as _st:
                const = _st.enter_context(tc.tile_pool(name="const", bufs=1))
                xres_pool = _st.enter_context(tc.tile_pool(name="xres", bufs=CT))
                yT_pool = _st.enter_context(tc.tile_pool(name="yTp", bufs=2))
                sb_stat = _st.enter_context(tc.tile_pool(name="stat", bufs=1))
                dram = _st.enter_context(tc.tile_pool(name="dram", bufs=1, space="DRAM"))
                w1_pool = _st.enter_context(tc.tile_pool(name="w1p", bufs=FT))
                # ---- constants ----
                ones_col = const.tile([128, 1], BF16, tag="ones_col")
                _dma(nc, ones_col[:], _col(ones_in[:]))
                ones_row = const.tile([1, 128], BF16, tag="ones_row")
                _dma(nc, ones_row[:], _row(ones_in[:]))
                # packed per-partition bias columns:
                # ln1g ln1b ln2g ln2b projb fc2b (8 each) fc1b (32)
                # qb_q (2) qb_k (2) ones (12) -> 96 cols
                bp = const.tile([128, 96], F32, tag="bp")
                _dma(nc, bp[:], bias_pack[:, :])
                ln1b_t = bp[:, 8:16]
                ln2b_t = bp[:, 24:32]
                projb_t = bp[:, 32:40]
                fc2b_t = bp[:, 40:48]
                fc1b_t = bp[:, 48:80]
                qb_q = bp[:, 80:82]
                qb_k = bp[:, 82:84]
                ones8 = bp[:, 84:92]
                s32 = bp[:, 92:93]
                gp_row = const.tile([1, 2 * DIM], BF16, tag="gp_row")
                _dma(nc, gp_row[:], _row(g_rows[:]))

                xres = [xres_pool.tile([128, NQ], BF16, tag="xres", name=f"xres{i}")
                        for i in range(CT)]
                yT = [yT_pool.tile([128, NTOK], BF16, tag="yT", name=f"yT{i}")
                      for i in range(2)]

                # ReduceScatter in two feature-halves so the 2nd overlaps LN2
                rs_in = [dram.tile([4 * DIM // 2, NQ], BF16, name=f"rs_in{i}")
                         for i in range(2)]
                rs_out = [dram.tile([DIM // 2, NQ], BF16, name=f"rs_out{i}")
                          for i in range(2)]

                # fc1 weights prefetched whole (8MB bf16 = 64KB sbuf): DMA
                # streams during LN1/attention; MLP never waits on fc1 loads.
                w1_t = [w1_pool.tile([128, 2, 512], FP8, tag="w1", name=f"w1_{i}")
                        for i in range(FT)]

                with tc.tile_pool(name="h1p", bufs=CT) as h1_pool:
                    h1 = [h1_pool.tile([128, NTOK], BF16, tag="h1", name=f"h1_{i}")
                          for i in range(CT)]

                    # ==== stage A+B: LN1 + Q/K/V, pipelined per 512-tok chunk ====
                    with ExitStack() as _stb:
                        kv_pool = _stb.enter_context(tc.tile_pool(name="kvp", bufs=1))
                        _stx = _stb.enter_context(ExitStack())
                        x_pool1 = _stx.enter_context(tc.tile_pool(name="xp1", bufs=CT))
                        ln_work = _stx.enter_context(tc.tile_pool(name="lnw", bufs=2))
                        w_pool = _stx.enter_context(tc.tile_pool(name="wp", bufs=3 * CT))
                        kT = [kv_pool.tile([128, NTOK], BF16, tag=f"kT{t}", name=f"kT{t}")
                              for t in range(2)]
                        qT = [kv_pool.tile([128, NTOK], BF16, tag=f"qT{t}", name=f"qT{t}")
                              for t in range(2)]
                        vaug = [kv_pool.tile([128, HC, 65], BF16, tag=f"va{t}", name=f"va{t}")
                                for t in range(16)]
                        x_t = [x_pool1.tile([128, NTOK], BF16, tag="xs", name=f"xs{i}")
                               for i in range(CT)]
                        wk_t = [w_pool.tile([128, GW], BF16, tag="wqkv", name=f"wk{i}")
                                for i in range(CT)]
                        wq_t = [w_pool.tile([128, GW], BF16, tag="wqkv", name=f"wq{i}")
                                for i in range(CT)]
                        wv_t = [w_pool.tile([128, GW], BF16, tag="wqkv", name=f"wv{i}")
                                for i in range(CT)]
                        with ExitStack() as _sta:
                            sq_pool = _sta.enter_context(tc.tile_pool(name="sqp", bufs=2))
                            psA = _sta.enter_context(tc.tile_pool(name="psA", bufs=4, space="PSUM"))
                            mu_ps = [psA.tile([1, NQ], F32, tag="mu", name=f"mu{i}")
                                     for i in range(4)]
                            musq_ps = [psA.tile([1, NQ], F32, tag="musq", name=f"musq{i}")
                                       for i in range(4)]
                            # chunk-major: chunk 0 stats complete after 1/4 of DMA
                            stats = []
                            for ch in range(4):
                                for ct in range(CT):
                                    _dma(nc, x_t[ct][:, ch * NQ : (ch + 1) * NQ],
                                         xT[ct * 128 : (ct + 1) * 128,
                                            ch * NQ : (ch + 1) * NQ])
                                if ch == 1:
                                    # qkv weights: behind chunk-0/1 x loads
                                    for ct in range(CT):
                                        rsl = slice(ct * 128, (ct + 1) * 128)
                                        _dma(nc, wk_t[ct][:], wkT[rsl, :])
                                        _dma(nc, wq_t[ct][:], wqT[rsl, :])
                                        _dma(nc, wv_t[ct][:], wvT[rsl, :])
                                for ct in range(CT):
                                    csl = slice(ch * NQ, (ch + 1) * NQ)
                                    sq = sq_pool.tile([128, NQ], BF16, tag="sq", name="sq")
                                    nc.vector.tensor_mul(sq[:], x_t[ct][:, csl],
                                                         x_t[ct][:, csl])
                                    nc.tensor.matmul(
                                        mu_ps[ch][:], ones_col[:], x_t[ct][:, csl],
                                        start=(ct == 0), stop=(ct == CT - 1))
                                    nc.tensor.matmul(
                                        musq_ps[ch][:], ones_col[:], sq[:],
                                        start=(ct == 0), stop=(ct == CT - 1))
                                stats.append(_ln_stats(nc, sb_stat, mu_ps[ch],
                                                       musq_ps[ch], NQ, DIM))
                            # fc1 prefetch (all of it, fp8 pair layout)
                            for i in range(FT):
                                fog, cp = i // 4, i % 4
                                _dma(nc, w1_t[i][:],
                                     fc1_w[cp * 128 : (cp + 1) * 128, :,
                                           fog * 512 : (fog + 1) * 512])
                        # per chunk: LN1 pass 2, then K/Q/V of that chunk
                        with ExitStack() as _stp:
                            psAb = _stp.enter_context(tc.tile_pool(name="psAb", bufs=3, space="PSUM"))
                            psB = _stp.enter_context(tc.tile_pool(name="psB", bufs=2, space="PSUM"))
                            for ch in range(4):
                                _ln_qkv_chunk(nc, psAb, psB, ln_work, gp_row,
                                              ln1b_t, qb_q, qb_k, ones8, stats,
                                              x_t, h1, wk_t, wq_t, wv_t,
                                              kT, qT, vaug, ch)
                        _stx.close()  # free x/weights sbuf before attention pools

                        # === stage C+D: attention with proj interleaved per qch ===
                        with ExitStack() as _stc:
                            e_pool = _stc.enter_context(tc.tile_pool(name="ep", bufs=4))
                            au_pool = _stc.enter_context(tc.tile_pool(name="aup", bufs=4))
                            pw_pool = _stc.enter_context(tc.tile_pool(name="pwp", bufs=2))
                            d_sb = _stc.enter_context(tc.tile_pool(name="dsb", bufs=4))
                            psS = _stc.enter_context(tc.tile_pool(name="psS", bufs=2, space="PSUM"))
                            psAV = _stc.enter_context(tc.tile_pool(name="psAV", bufs=2, space="PSUM"))
                            psRB = _stc.enter_context(tc.tile_pool(name="psRB", bufs=1, space="PSUM"))
                            psD = _stc.enter_context(tc.tile_pool(name="psD", bufs=1, space="PSUM"))
                            pw_t = [pw_pool.tile([128, DIM], BF16, tag="pw", name=f"pw{i}")
                                    for i in range(2)]
                            for t in range(2):
                                _dma(nc, pw_t[t][:], proj_wT[t * 128 : (t + 1) * 128, :])
                            for qch in range(4):
                                qsl = slice(qch * NQ, (qch + 1) * NQ)
                                for t in range(2):
                                    _attn_unit(nc, psS, psAV, psRB, e_pool,
                                               au_pool, sb_stat, kT, qT, vaug,
                                               yT, ones_row, t, qsl)
                                for co in range(CT):
                                    _proj_co(nc, psD, d_sb, pw_t, yT, rs_in,
                                             qch, co)
                            for half in range(2):
                                nc.gpsimd.collective_compute(
                                    "ReduceScatter", mybir.AluOpType.add,
                                    replica_groups=REPLICA_GROUPS,
                                    ins=[rs_in[half].opt()],
                                    outs=[rs_out[half].opt()])


                # ====== stage E: residual + LN2 on own 512-token block ======
                with tc.tile_pool(name="x2p", bufs=CT) as x2_pool:
                    x2 = [x2_pool.tile([128, NQ], BF16, tag="x2", name=f"x2_{i}")
                          for i in range(CT)]
                    with tc.tile_pool(name="h2p", bufs=CT) as h2_pool:
                        h2_8 = [h2_pool.tile([128, 2, NQ], FP8, tag="h2", name=f"h2_{i}")
                                for i in range(4)]
                        x2b = [h2_pool.tile([128, NQ], BF16, tag="x2b", name=f"x2b{i}")
                               for i in range(CT)]
                        with ExitStack() as _ste:
                            rs_pool = _ste.enter_context(tc.tile_pool(name="rsp", bufs=CT))
                            psE = _ste.enter_context(tc.tile_pool(name="psE", bufs=1, space="PSUM"))
                            psEb = _ste.enter_context(tc.tile_pool(name="psEb", bufs=2, space="PSUM"))
                            sq2_pool = _ste.enter_context(tc.tile_pool(name="sq2p", bufs=2))
                            ln_work2 = _ste.enter_context(tc.tile_pool(name="lnw2", bufs=2))
                            for ct in range(CT):
                                _dma(nc, xres[ct][:],
                                     xresT[ct * 128 : (ct + 1) * 128, :])
                            rs_t = [rs_pool.tile([128, NQ], BF16, tag="rs", name=f"rs{i}")
                                    for i in range(CT)]
                            for ct in range(CT):
                                _dma(nc, rs_t[ct][:],
                                     rs_out[ct // 4][(ct % 4) * 128
                                                     : (ct % 4 + 1) * 128, :])
                            mu_ps = psE.tile([1, NQ], F32, tag="mu2", name="mu2")
                            musq_ps = psE.tile([1, NQ], F32, tag="musq2", name="musq2")
                            for co in range(CT):
                                nc.vector.scalar_tensor_tensor(
                                    x2[co][:], rs_t[co][:], projb_t[:, co : co + 1],
                                    xres[co][:], op0=ALU.add, op1=ALU.add)
                                nc.vector.tensor_scalar_add(
                                    x2b[co][:], x2[co][:], fc2b_t[:, co : co + 1])
                                sq = sq2_pool.tile([128, NQ], BF16, tag="sq2", name="sq2")
                                nc.vector.tensor_mul(sq[:], x2[co][:], x2[co][:])
                                nc.tensor.matmul(mu_ps[:], ones_col[:], x2[co][:],
                                                 start=(co == 0), stop=(co == CT - 1))
                                nc.tensor.matmul(musq_ps[:], ones_col[:], sq[:],
                                                 start=(co == 0), stop=(co == CT - 1))
                            rm = _ln_stats(nc, sb_stat, mu_ps, musq_ps, NQ, DIM)
                            for ct in range(CT):
                                grow = gp_row[0:1, DIM + ct * 128 : DIM + (ct + 1) * 128]
                                bc = psEb.tile([128, 2 * NQ], F32, tag="bc2", name="bc2")
                                nc.tensor.matmul(bc[:, 0:NQ], grow,
                                                 rm[:, 0:NQ],
                                                 start=True, stop=True)
                                nc.tensor.matmul(bc[:, NQ : 2 * NQ], grow,
                                                 rm[:, NQ : 2 * NQ],
                                                 start=True, stop=True)
                                t = ln_work2.tile([128, NQ], BF16, tag="lnt2", name="lnt2")
                                nc.vector.tensor_mul(t[:], x2[ct][:], bc[:, 0:NQ])
                                nc.vector.scalar_tensor_tensor(
                                    h2_8[ct // 2][:, ct % 2, :], t[:],
                                    ln2b_t[:, ct : ct + 1],
                                    bc[:, NQ : 2 * NQ], op0=ALU.add, op1=ALU.subtract)

                        # ============ stage F: MLP ==============================
                        with tc.tile_pool(name="gp", bufs=FT) as g_pool:
                            g_t = [g_pool.tile([128, NQ], BF16, tag="g", name=f"g{i}")
                                   for i in range(FT)]
                            with tc.tile_pool(name="psF1", bufs=8, space="PSUM") as psF1:
                                for fog in range(8):
                                    pss = [psF1.tile([128, NQ], F32, tag="f1ps",
                                                     name=f"f1ps{i}")
                                           for i in range(4)]
                                    for cp in range(4):
                                        for fo4 in range(4):
                                            nc.tensor.matmul(
                                                pss[fo4][:],
                                                w1_t[fog * 4 + cp][
                                                    :, :, fo4 * 128 : (fo4 + 1) * 128],
                                                h2_8[cp][:],
                                                start=(cp == 0), stop=(cp == 3),
                                                perf_mode=mybir.MatmulPerfMode.DoubleRow)
                                    for fo4 in range(4):
                                        fo = fog * 4 + fo4
                                        nc.scalar.activation(
                                            g_t[fo][:], pss[fo4][:],
                                            GELU_AF or AF.Gelu,
                                            bias=fc1b_t[:, fo : fo + 1],
                                            scale=1.0 / W8SCALE)
                            # fc2 + residual
                            with ExitStack() as _stf:
                                w2_pool = _stf.enter_context(tc.tile_pool(name="w2p", bufs=24))
                                psF2 = _stf.enter_context(tc.tile_pool(name="psF2", bufs=8, space="PSUM"))
                                out_pool = _stf.enter_context(tc.tile_pool(name="op", bufs=4))
                                for cog in range(2):
                                    pss = [psF2.tile([128, NQ], F32, tag="f2ps",
                                                     name=f"f2ps{i}")
                                           for i in range(4)]
                                    for ko in range(FT):
                                        w2_t = w2_pool.tile([128, 512], BF16, tag="w2")
                                        _dma(nc, w2_t[:],
                                             fc2_w[ko * 128 : (ko + 1) * 128,
                                                   cog * 512 : (cog + 1) * 512])
                                        for co4 in range(4):
                                            nc.tensor.matmul(
                                                pss[co4][:],
                                                w2_t[:, co4 * 128 : (co4 + 1) * 128],
                                                g_t[ko][:],
                                                start=(ko == 0), stop=(ko == FT - 1))
                                    for co4 in range(4):
                                        co = cog * 4 + co4
                                        o_t = out_pool.tile([128, NQ], F32, tag="o")
                                        nc.vector.scalar_tensor_tensor(
                                            o_t[:], pss[co4][:], fc2b_t[:, co : co + 1],
                                            x2[co][:], op0=ALU.add, op1=ALU.add)
                                        _dma(nc, outT[co * 128 : (co + 1) * 128, :], o_t[:])


_CACHED_NC = None


def _get_nc():
    global _CACHED_NC
    if _CACHED_NC is None:
        _CACHED_NC = build_program()
    return _CACHED_NC


def make_in_maps(inputs):
    from ml_dtypes import bfloat16

    ins = {k: np.ascontiguousarray(np.asarray(v), dtype=np.float32)
           for k, v in inputs.items()}
    proj_b_eff = (ins["proj_b"]
                  + ins["qkv_b"][2048:].astype(np.float64)
                  @ ins["proj_w"].astype(np.float64)).astype(np.float32)
    from ml_dtypes import float8_e4m3
    fc1_wb = np.ascontiguousarray(
        (ins["fc1_w"].reshape(4, 2, 128, MLP).transpose(0, 2, 1, 3)
         * W8SCALE).reshape(DIM // 2, 2, MLP)).astype(float8_e4m3)
    fc2_wb = np.ascontiguousarray(ins["fc2_w"]).astype(bfloat16)
    g_rows = np.concatenate([ins["ln1_g"], ins["ln2_g"]]).astype(bfloat16)
    in_maps = []
    for core in range(N_CORES):
        b = core // 4
        r = core % 4
        hsl = slice(r * GW, (r + 1) * GW)
        cols = [ins["ln1_g"], ins["ln1_b"], ins["ln2_g"], ins["ln2_b"],
                proj_b_eff, ins["fc2_b"], ins["fc1_b"],
                ins["qkv_b"][hsl], ins["qkv_b"][1024 + r * GW : 1024 + (r + 1) * GW]]
        packed = np.concatenate(
            [c.reshape(-1, 128).T for c in cols]
            + [np.ones((128, 8), np.float32),
               np.full((128, 1), 1.0 / W8SCALE, np.float32),
               np.ones((128, 3), np.float32)],
            axis=1)
        in_maps.append({
            "xT": np.ascontiguousarray(ins["x"][b].T).astype(bfloat16),
            "xresT": np.ascontiguousarray(
                ins["x"][b][r * NQ : (r + 1) * NQ, :].T).astype(bfloat16),
            "wqT": np.ascontiguousarray(ins["qkv_w"][:, hsl]).astype(bfloat16),
            "wkT": np.ascontiguousarray(
                ins["qkv_w"][:, 1024 + r * GW : 1024 + (r + 1) * GW]).astype(bfloat16),
            "wvT": np.ascontiguousarray(
                ins["qkv_w"][:, 2048 + r * GW : 2048 + (r + 1) * GW]).astype(bfloat16),
            "proj_wT": np.ascontiguousarray(ins["proj_w"][hsl, :]).astype(bfloat16),
            "fc1_w": fc1_wb,
            "fc2_w": fc2_wb,
            "ones_in": np.ones(128, bfloat16),
            "bias_pack": np.ascontiguousarray(packed),
            "g_rows": g_rows,
        })
    return in_maps


def gather_output(results):
    out = np.empty((2, NTOK, DIM), dtype=np.float32)
    for core in range(N_CORES):
        b = core // 4
        qs = (core % 4) * NQ
        out[b, qs : qs + NQ, :] = results[core]["outT"].T
    return out


def kernel(**inputs) -> np.ndarray:
    nc = _get_nc()
    in_maps = make_in_maps(inputs)
    res = run_bass_kernel_spmd(nc, in_maps, list(range(N_CORES)))
    return gather_output(res.results)


if __name__ == "__main__":
    rng = np.random.default_rng(0)
    demo = {
        "x": rng.standard_normal((2, NTOK, DIM), dtype=np.float32),
        "ln1_g": np.ones(DIM, np.float32), "ln1_b": np.zeros(DIM, np.float32),
        "qkv_w": (rng.standard_normal((DIM, 3 * DIM)) * 0.02).astype(np.float32),
        "qkv_b": np.zeros(3 * DIM, np.float32),
        "proj_w": (rng.standard_normal((DIM, DIM)) * 0.02).astype(np.float32),
        "proj_b": np.zeros(DIM, np.float32),
        "ln2_g": np.ones(DIM, np.float32), "ln2_b": np.zeros(DIM, np.float32),
        "fc1_w": (rng.standard_normal((DIM, MLP)) * 0.02).astype(np.float32),
        "fc1_b": np.zeros(MLP, np.float32),
        "fc2_w": (rng.standard_normal((MLP, DIM)) * 0.02).astype(np.float32),
        "fc2_b": np.zeros(DIM, np.float32),
    }
    out = kernel(**demo)
    print("out", out.shape, out.dtype, float(np.abs(out).max()))


# revision 29
# speedup vs baseline: 203.3647x; 1.0311x over previous
"""Trainium2 Bass kernel for a dense pre-norm transformer block.

Reference computation (fp32):
    h = LN1(x); qkv = h @ qkv_w + qkv_b; attention (16 heads, no 1/sqrt(d));
    x = x + attn_out @ proj_w + proj_b;
    h2 = LN2(x); x = x + gelu_exact(h2 @ fc1_w + fc1_b) @ fc2_w + fc2_b

Shapes: x [2, 2048, 1024], heads 16 x 64, MLP 4096.

Sharding (8 NeuronCores, Megatron-style tensor parallel over heads):
    cores 0-3 -> batch 0, cores 4-7 -> batch 1. Within a 4-core group,
    core r owns HEADS 4r..4r+3 for attention (Q/K/V/scores/AV computed for
    those heads over ALL 2048 tokens -> no replicated K/V GEMMs), then a
    head-partial projection produces partial x2 for all tokens, which a
    bf16 ReduceScatter(add) over the group turns into the full proj output
    for the core's OWN 512-token block. LN2 + MLP + residual run on that
    block only. Everything is absolute token order -> one uniform SPMD
    program; per-core differences live in host-sliced weights.

Layout: activations feature-major [C, tokens]; all matmul operands bf16
    (fp32 PSUM accumulation); LN stats / softmax sums via ones-vector
    matmuls; softmax sums ride the attn@V matmul as a 65th V column; exp
    batched over two PSUM banks per instruction to amortize ACT overhead.
"""

import sys

if "/opt/trn_rl_repo" not in sys.path:
    sys.path.insert(0, "/opt/trn_rl_repo")

from contextlib import ExitStack

import numpy as np

import concourse.bass as bass
import concourse.mybir as mybir
import concourse.tile as tile
from concourse import bacc
from concourse.bass_utils import run_bass_kernel_spmd

F32 = mybir.dt.float32
BF16 = mybir.dt.bfloat16
F32R = mybir.dt.float32r
FP8 = mybir.dt.float8e4
W8SCALE = 32.0
AF = mybir.ActivationFunctionType
ALU = mybir.AluOpType

DIM = 1024
CT = DIM // 128          # 8 feature tiles
NTOK = 2048              # tokens per batch
NQ = 512                 # own token block (proj output / MLP)
H = 16
HC = 4                   # heads per core
D = 64
GW = HC * D              # 256 qkv columns per core
MLP = 4096
FT = MLP // 128          # 32 mlp feature tiles
EPS = 1e-5
N_CORES = 8
GELU_AF = None  # test hook: set to AF.Identity to bypass gelu in CoreSim
REPLICA_GROUPS = [[0, 1, 2, 3], [4, 5, 6, 7]]


def _dma(nc, out, in_):
    nc.sync.dma_start(out=out, in_=in_)


def _col(v):
    return v.rearrange("(p o) -> p o", o=1)


def _row(v):
    return v.rearrange("(o f) -> o f", o=1)


def _ln_stats(nc, sb_stat, mu_ps, musq_ps, n, ntok_norm):
    """From accumulated sum / sum-of-squares psums [1, n] produce
    rstd [1,n] and mean*rstd [1,n] (bf16 sbuf, packed in one tile)."""
    mean = sb_stat.tile([1, n], F32, tag="mean", bufs=1, name="mean")
    w = sb_stat.tile([1, n], F32, tag="w", bufs=1, name="w")
    nc.vector.tensor_scalar_mul(mean[:], mu_ps[:], 1.0 / ntok_norm)
    nc.vector.tensor_scalar_mul(w[:], musq_ps[:], 1.0 / ntok_norm)
    m2 = sb_stat.tile([1, n], F32, tag="m2", bufs=1, name="m2")
    nc.vector.tensor_mul(m2[:], mean[:], mean[:])
    nc.vector.tensor_sub(w[:], w[:], m2[:])
    nc.vector.tensor_scalar_add(w[:], w[:], EPS)
    nc.vector.reciprocal(m2[:], w[:])
    rm = sb_stat.tile([1, 2 * n], BF16, tag="rm", bufs=4, name="rm")
    rstd = rm[:, 0:n]
    mrs = rm[:, n : 2 * n]
    nc.scalar.activation(rstd, m2[:], AF.Sqrt)
    nc.vector.tensor_mul(mrs, mean[:], rstd)
    return rm


def _proj_co(nc, psD, d_sb, pw_t, yT, rs_in, qch, co):
    """One 128-feature column of the head-partial projection for block qch."""
    ps = psD.tile([128, NQ], F32, tag="dps", name="dps")
    qsl = slice(qch * NQ, (qch + 1) * NQ)
    for t in range(2):
        nc.tensor.matmul(ps[:], pw_t[t][:, co * 128 : (co + 1) * 128],
                         yT[t][:, qsl], start=(t == 0), stop=(t == 1))
    stg = d_sb.tile([128, NQ], BF16, tag="stg", name="stg")
    nc.vector.tensor_copy(stg[:], ps[:])
    half, cof = co // 4, co % 4
    nc.sync.dma_start(
        out=rs_in[half][qch * NQ + cof * 128 : qch * NQ + (cof + 1) * 128, :],
        in_=stg[:])


def _attn_unit(nc, psS, psAV, psRB, e_pool, au_pool, sb_stat,
               kT, qT, vaug, yT, ones_row, t, qsl, side=None):
    """Attention for one (head-pair tile t, 512-query chunk): scores ->
    batched exp -> attn@V (one kt behind) -> softmax normalize into yT."""
    av2 = [psAV.tile([65, NQ], F32, tag="av", name=f"av{h}") for h in range(2)]
    prev_e = None
    for kt in range(16):
        s2 = psS.tile([128, 2 * NQ], F32, tag="s", name="s")
        for hh in range(2):
            hsl = slice(hh * 64, (hh + 1) * 64)
            nc.tensor.matmul(
                s2[:, hh * NQ : (hh + 1) * NQ],
                kT[t][hsl, kt * 128 : (kt + 1) * 128],
                qT[t][hsl, qsl],
                start=True, stop=True)
        e_t = e_pool.tile([128, 2 * NQ], F32R, tag="e", name="e")
        nc.scalar.activation(e_t[:], s2[:], AF.Exp)
        if prev_e is not None:
            for hh in range(2):
                nc.tensor.matmul(
                    av2[hh][:], vaug[kt - 1][:, 2 * t + hh, :],
                    prev_e[:, hh * NQ : (hh + 1) * NQ],
                    start=(kt == 1), stop=False)
        if side is not None and 1 <= kt <= len(side):
            side[kt - 1]()  # interleave prev block's proj between score streams
        prev_e = e_t
    for hh in range(2):
        nc.tensor.matmul(
            av2[hh][:], vaug[15][:, 2 * t + hh, :],
            prev_e[:, hh * NQ : (hh + 1) * NQ],
            start=False, stop=True)
        au = au_pool.tile([65, NQ], F32, tag="au", name="au")
        nc.vector.tensor_copy(au[:], av2[hh][:])
        rcp = sb_stat.tile([1, NQ], BF16, tag="rcp", bufs=2, name="rcp")
        with nc.allow_low_precision("softmax 1/sum"):
            nc.vector.reciprocal(rcp[:], au[64:65, :])
        rb = psRB.tile([64, NQ], F32, tag="rb", name="rb")
        nc.tensor.matmul(rb[:], ones_row[:, 0:64], rcp[:], start=True, stop=True)
        nc.vector.tensor_mul(yT[t][hh * 64 : (hh + 1) * 64, qsl],
                             au[0:64, :], rb[:])


def _ln_qkv_chunk(nc, psAb, psB, ln_work, gp_row, ln1b_t, qb_q, qb_k, ones8,
                  stats, x_t, h1, wk_t, wq_t, wv_t, kT, qT, vaug, ch):
    """LN1 pass 2 for one 512-token chunk, then K/Q/V of that chunk."""
    csl = slice(ch * NQ, (ch + 1) * NQ)
    rm = stats[ch]
    for ct in range(CT):
        grow = gp_row[0:1, ct * 128 : (ct + 1) * 128]
        bc = psAb.tile([128, 2 * NQ], F32, tag="bc", name="bc")
        nc.tensor.matmul(bc[:, 0:NQ], grow, rm[:, 0:NQ], start=True, stop=True)
        nc.tensor.matmul(bc[:, NQ : 2 * NQ], grow, rm[:, NQ : 2 * NQ],
                         start=True, stop=True)
        t = ln_work.tile([128, NQ], BF16, tag="lnt", name="lnt")
        nc.vector.tensor_mul(t[:], x_t[ct][:, csl], bc[:, 0:NQ])
        nc.vector.scalar_tensor_tensor(
            h1[ct][:, csl], t[:], ln1b_t[:, ct : ct + 1],
            bc[:, NQ : 2 * NQ], op0=ALU.add, op1=ALU.subtract)
    # K/Q for this chunk
    for t in range(2):
        tsl = slice(t * 128, (t + 1) * 128)
        ps = psB.tile([128, NQ], F32, tag="bps", name="bps")
        for ct in range(CT):
            nc.tensor.matmul(ps[:], wk_t[ct][:, tsl], h1[ct][:, csl],
                             start=(ct == 0), stop=(ct == CT - 1))
        nc.vector.tensor_scalar_add(kT[t][:, csl], ps[:], qb_k[:, t : t + 1])
        ps = psB.tile([128, NQ], F32, tag="bps", name="bps")
        for ct in range(CT):
            nc.tensor.matmul(ps[:], wq_t[ct][:, tsl], h1[ct][:, csl],
                             start=(ct == 0), stop=(ct == CT - 1))
        nc.vector.tensor_scalar_add(qT[t][:, csl], ps[:], qb_q[:, t : t + 1])
    # V for this chunk's 4 token-tiles
    for tt in range(4 * ch, 4 * ch + 4):
        tsl = slice(tt * 128, (tt + 1) * 128)
        ps = psB.tile([128, GW], F32, tag="bps", name="vps")
        for ct in range(CT):
            nc.tensor.matmul(ps[:], h1[ct][:, tsl], wv_t[ct][:],
                             start=(ct == 0), stop=(ct == CT - 1))
        nc.vector.tensor_copy(vaug[tt][:, :, 0:64],
                              ps[:].rearrange("p (a f) -> p a f", f=64))
        nc.vector.tensor_copy(vaug[tt][:, :, 64:65],
                              ones8[:, 0:HC].rearrange("p (a o) -> p a o", o=1))


def build_program(reps=1):
    nc = bacc.Bacc("TRN2", target_bir_lowering=False)

    xT = nc.declare_dram_parameter("xT", [DIM, NTOK], BF16, isOutput=False)
    xresT = nc.declare_dram_parameter("xresT", [DIM, NQ], BF16, isOutput=False)
    wqT = nc.declare_dram_parameter("wqT", [DIM, GW], BF16, isOutput=False)
    wkT = nc.declare_dram_parameter("wkT", [DIM, GW], BF16, isOutput=False)
    wvT = nc.declare_dram_parameter("wvT", [DIM, GW], BF16, isOutput=False)
    proj_wT = nc.declare_dram_parameter("proj_wT", [GW, DIM], BF16, isOutput=False)
    fc1_w = nc.declare_dram_parameter("fc1_w", [DIM // 2, 2, MLP], FP8, isOutput=False)
    fc2_w = nc.declare_dram_parameter("fc2_w", [MLP, DIM], BF16, isOutput=False)
    ones_in = nc.declare_dram_parameter("ones_in", [128], BF16, isOutput=False)
    bias_pack = nc.declare_dram_parameter("bias_pack", [128, 96], F32, isOutput=False)
    g_rows = nc.declare_dram_parameter("g_rows", [2 * DIM], BF16, isOutput=False)
    outT = nc.declare_dram_parameter("outT", [DIM, NQ], F32, isOutput=True)

    with tile.TileContext(nc, pool_alloc_mode="queue") as tc:
        for _rep in range(reps):
            _build_rep(nc, tc, xT, xresT, wqT, wkT, wvT, proj_wT, fc1_w, fc2_w,
                       ones_in, bias_pack, g_rows, outT)

    nc.compile()
    return nc


def _build_rep(nc, tc, xT, xresT, wqT, wkT, wvT, proj_wT, fc1_w, fc2_w,
               ones_in, bias_pack, g_rows, outT):
            with ExitStack() as _st:
                const = _st.enter_context(tc.tile_pool(name="const", bufs=1))
                xres_pool = _st.enter_context(tc.tile_pool(name="xres", bufs=CT))
                yT_pool = _st.enter_context(tc.tile_pool(name="yTp", bufs=2))
                sb_stat = _st.enter_context(tc.tile_pool(name="stat", bufs=1))
                dram = _st.enter_context(tc.tile_pool(name="dram", bufs=1, space="DRAM"))
                w1_pool = _st.enter_context(tc.tile_pool(name="w1p", bufs=FT))
                # ---- constants ----
                ones_col = const.tile([128, 1], BF16, tag="ones_col")
                _dma(nc, ones_col[:], _col(ones_in[:]))
                ones_row = const.tile([1, 128], BF16, tag="ones_row")
                _dma(nc, ones_row[:], _row(ones_in[:]))
                # packed per-partition bias columns:
                # ln1g ln1b ln2g ln2b projb fc2b (8 each) fc1b (32)
                # qb_q (2) qb_k (2) ones (12) -> 96 cols
                bp = const.tile([128, 96], F32, tag="bp")
                _dma(nc, bp[:], bias_pack[:, :])
                ln1b_t = bp[:, 8:16]
                ln2b_t = bp[:, 24:32]
                projb_t = bp[:, 32:40]
                fc2b_t = bp[:, 40:48]
                fc1b_t = bp[:, 48:80]
                qb_q = bp[:, 80:82]
                qb_k = bp[:, 82:84]
                ones8 = bp[:, 84:92]
                s32 = bp[:, 92:93]
                gp_row = const.tile([1, 2 * DIM], BF16, tag="gp_row")
                _dma(nc, gp_row[:], _row(g_rows[:]))

                xres = [xres_pool.tile([128, NQ], BF16, tag="xres", name=f"xres{i}")
                        for i in range(CT)]
                yT = [yT_pool.tile([128, NTOK], BF16, tag="yT", name=f"yT{i}")
                      for i in range(2)]

                # ReduceScatter in two feature-halves so the 2nd overlaps LN2
                rs_in = [dram.tile([4 * DIM // 2, NQ], BF16, name=f"rs_in{i}")
                         for i in range(2)]
                rs_out = [dram.tile([DIM // 2, NQ], BF16, name=f"rs_out{i}")
                          for i in range(2)]

                # fc1 weights prefetched whole (8MB bf16 = 64KB sbuf): DMA
                # streams during LN1/attention; MLP never waits on fc1 loads.
                w1_t = [w1_pool.tile([128, 2, 512], FP8, tag="w1", name=f"w1_{i}")
                        for i in range(FT)]

                with tc.tile_pool(name="h1p", bufs=CT) as h1_pool:
                    h1 = [h1_pool.tile([128, NTOK], BF16, tag="h1", name=f"h1_{i}")
                          for i in range(CT)]

                    # ==== stage A+B: LN1 + Q/K/V, pipelined per 512-tok chunk ====
                    with ExitStack() as _stb:
                        kv_pool = _stb.enter_context(tc.tile_pool(name="kvp", bufs=1))
                        _stx = _stb.enter_context(ExitStack())
                        x_pool1 = _stx.enter_context(tc.tile_pool(name="xp1", bufs=CT))
                        ln_work = _stx.enter_context(tc.tile_pool(name="lnw", bufs=2))
                        w_pool = _stx.enter_context(tc.tile_pool(name="wp", bufs=3 * CT))
                        kT = [kv_pool.tile([128, NTOK], BF16, tag=f"kT{t}", name=f"kT{t}")
                              for t in range(2)]
                        qT = [kv_pool.tile([128, NTOK], BF16, tag=f"qT{t}", name=f"qT{t}")
                              for t in range(2)]
                        vaug = [kv_pool.tile([128, HC, 65], F32R, tag=f"va{t}", name=f"va{t}")
                                for t in range(16)]
                        x_t = [x_pool1.tile([128, NTOK], BF16, tag="xs", name=f"xs{i}")
                               for i in range(CT)]
                        wk_t = [w_pool.tile([128, GW], BF16, tag="wqkv", name=f"wk{i}")
                                for i in range(CT)]
                        wq_t = [w_pool.tile([128, GW], BF16, tag="wqkv", name=f"wq{i}")
                                for i in range(CT)]
                        wv_t = [w_pool.tile([128, GW], BF16, tag="wqkv", name=f"wv{i}")
                                for i in range(CT)]
                        with ExitStack() as _sta:
                            sq_pool = _sta.enter_context(tc.tile_pool(name="sqp", bufs=2))
                            psA = _sta.enter_context(tc.tile_pool(name="psA", bufs=4, space="PSUM"))
                            mu_ps = [psA.tile([1, NQ], F32, tag="mu", name=f"mu{i}")
                                     for i in range(4)]
                            musq_ps = [psA.tile([1, NQ], F32, tag="musq", name=f"musq{i}")
                                       for i in range(4)]
                            # chunk-major: chunk 0 stats complete after 1/4 of DMA
                            stats = []
                            for ch in range(4):
                                for ct in range(CT):
                                    _dma(nc, x_t[ct][:, ch * NQ : (ch + 1) * NQ],
                                         xT[ct * 128 : (ct + 1) * 128,
                                            ch * NQ : (ch + 1) * NQ])
                                if ch == 1:
                                    # qkv weights: behind chunk-0/1 x loads
                                    for ct in range(CT):
                                        rsl = slice(ct * 128, (ct + 1) * 128)
                                        _dma(nc, wk_t[ct][:], wkT[rsl, :])
                                        _dma(nc, wq_t[ct][:], wqT[rsl, :])
                                        _dma(nc, wv_t[ct][:], wvT[rsl, :])
                                for ct in range(CT):
                                    csl = slice(ch * NQ, (ch + 1) * NQ)
                                    sq = sq_pool.tile([128, NQ], BF16, tag="sq", name="sq")
                                    nc.vector.tensor_mul(sq[:], x_t[ct][:, csl],
                                                         x_t[ct][:, csl])
                                    nc.tensor.matmul(
                                        mu_ps[ch][:], ones_col[:], x_t[ct][:, csl],
                                        start=(ct == 0), stop=(ct == CT - 1))
                                    nc.tensor.matmul(
                                        musq_ps[ch][:], ones_col[:], sq[:],
                                        start=(ct == 0), stop=(ct == CT - 1))
                                stats.append(_ln_stats(nc, sb_stat, mu_ps[ch],
                                                       musq_ps[ch], NQ, DIM))
                            # fc1 prefetch (all of it, fp8 pair layout)
                            for i in range(FT):
                                fog, cp = i // 4, i % 4
                                _dma(nc, w1_t[i][:],
                                     fc1_w[cp * 128 : (cp + 1) * 128, :,
                                           fog * 512 : (fog + 1) * 512])
                        # per chunk: LN1 pass 2, then K/Q/V of that chunk
                        with ExitStack() as _stp:
                            psAb = _stp.enter_context(tc.tile_pool(name="psAb", bufs=3, space="PSUM"))
                            psB = _stp.enter_context(tc.tile_pool(name="psB", bufs=2, space="PSUM"))
                            for ch in range(4):
                                _ln_qkv_chunk(nc, psAb, psB, ln_work, gp_row,
                                              ln1b_t, qb_q, qb_k, ones8, stats,
                                              x_t, h1, wk_t, wq_t, wv_t,
                                              kT, qT, vaug, ch)
                        _stx.close()  # free x/weights sbuf before attention pools

                        # === stage C+D: attention with proj interleaved per qch ===
                        with ExitStack() as _stc:
                            e_pool = _stc.enter_context(tc.tile_pool(name="ep", bufs=3))
                            au_pool = _stc.enter_context(tc.tile_pool(name="aup", bufs=4))
                            pw_pool = _stc.enter_context(tc.tile_pool(name="pwp", bufs=2))
                            d_sb = _stc.enter_context(tc.tile_pool(name="dsb", bufs=4))
                            psS = _stc.enter_context(tc.tile_pool(name="psS", bufs=2, space="PSUM"))
                            psAV = _stc.enter_context(tc.tile_pool(name="psAV", bufs=2, space="PSUM"))
                            psRB = _stc.enter_context(tc.tile_pool(name="psRB", bufs=1, space="PSUM"))
                            psD = _stc.enter_context(tc.tile_pool(name="psD", bufs=1, space="PSUM"))
                            pw_t = [pw_pool.tile([128, DIM], BF16, tag="pw", name=f"pw{i}")
                                    for i in range(2)]
                            for t in range(2):
                                _dma(nc, pw_t[t][:], proj_wT[t * 128 : (t + 1) * 128, :])
                            for qch in range(4):
                                qsl = slice(qch * NQ, (qch + 1) * NQ)
                                for t in range(2):
                                    _attn_unit(nc, psS, psAV, psRB, e_pool,
                                               au_pool, sb_stat, kT, qT, vaug,
                                               yT, ones_row, t, qsl)
                                for co in range(CT):
                                    _proj_co(nc, psD, d_sb, pw_t, yT, rs_in,
                                             qch, co)
                            for half in range(2):
                                nc.gpsimd.collective_compute(
                                    "ReduceScatter", mybir.AluOpType.add,
                                    replica_groups=REPLICA_GROUPS,
                                    ins=[rs_in[half].opt()],
                                    outs=[rs_out[half].opt()])


                # ====== stage E: residual + LN2 on own 512-token block ======
                with tc.tile_pool(name="x2p", bufs=CT) as x2_pool:
                    x2 = [x2_pool.tile([128, NQ], BF16, tag="x2", name=f"x2_{i}")
                          for i in range(CT)]
                    with tc.tile_pool(name="h2p", bufs=CT) as h2_pool:
                        h2_8 = [h2_pool.tile([128, 2, NQ], FP8, tag="h2", name=f"h2_{i}")
                                for i in range(4)]
                        x2b = [h2_pool.tile([128, NQ], BF16, tag="x2b", name=f"x2b{i}")
                               for i in range(CT)]
                        with ExitStack() as _ste:
                            rs_pool = _ste.enter_context(tc.tile_pool(name="rsp", bufs=CT))
                            psE = _ste.enter_context(tc.tile_pool(name="psE", bufs=1, space="PSUM"))
                            psEb = _ste.enter_context(tc.tile_pool(name="psEb", bufs=2, space="PSUM"))
                            sq2_pool = _ste.enter_context(tc.tile_pool(name="sq2p", bufs=2))
                            ln_work2 = _ste.enter_context(tc.tile_pool(name="lnw2", bufs=2))
                            for ct in range(CT):
                                _dma(nc, xres[ct][:],
                                     xresT[ct * 128 : (ct + 1) * 128, :])
                            rs_t = [rs_pool.tile([128, NQ], BF16, tag="rs", name=f"rs{i}")
                                    for i in range(CT)]
                            for ct in range(CT):
                                _dma(nc, rs_t[ct][:],
                                     rs_out[ct // 4][(ct % 4) * 128
                                                     : (ct % 4 + 1) * 128, :])
                            mu_ps = psE.tile([1, NQ], F32, tag="mu2", name="mu2")
                            musq_ps = psE.tile([1, NQ], F32, tag="musq2", name="musq2")
                            for co in range(CT):
                                nc.vector.scalar_tensor_tensor(
                                    x2[co][:], rs_t[co][:], projb_t[:, co : co + 1],
                                    xres[co][:], op0=ALU.add, op1=ALU.add)
                                nc.vector.tensor_scalar_add(
                                    x2b[co][:], x2[co][:], fc2b_t[:, co : co + 1])
                                sq = sq2_pool.tile([128, NQ], BF16, tag="sq2", name="sq2")
                                nc.vector.tensor_mul(sq[:], x2[co][:], x2[co][:])
                                nc.tensor.matmul(mu_ps[:], ones_col[:], x2[co][:],
                                                 start=(co == 0), stop=(co == CT - 1))
                                nc.tensor.matmul(musq_ps[:], ones_col[:], sq[:],
                                                 start=(co == 0), stop=(co == CT - 1))
                            rm = _ln_stats(nc, sb_stat, mu_ps, musq_ps, NQ, DIM)
                            for ct in range(CT):
                                grow = gp_row[0:1, DIM + ct * 128 : DIM + (ct + 1) * 128]
                                bc = psEb.tile([128, 2 * NQ], F32, tag="bc2", name="bc2")
                                nc.tensor.matmul(bc[:, 0:NQ], grow,
                                                 rm[:, 0:NQ],
                                                 start=True, stop=True)
                                nc.tensor.matmul(bc[:, NQ : 2 * NQ], grow,
                                                 rm[:, NQ : 2 * NQ],
                                                 start=True, stop=True)
                                t = ln_work2.tile([128, NQ], BF16, tag="lnt2", name="lnt2")
                                nc.vector.tensor_mul(t[:], x2[ct][:], bc[:, 0:NQ])
                                nc.vector.scalar_tensor_tensor(
                                    h2_8[ct // 2][:, ct % 2, :], t[:],
                                    ln2b_t[:, ct : ct + 1],
                                    bc[:, NQ : 2 * NQ], op0=ALU.add, op1=ALU.subtract)

                        # ============ stage F: MLP ==============================
                        with tc.tile_pool(name="gp", bufs=FT) as g_pool:
                            g_t = [g_pool.tile([128, NQ], BF16, tag="g", name=f"g{i}")
                                   for i in range(FT)]
                            with tc.tile_pool(name="psF1", bufs=8, space="PSUM") as psF1:
                                for fog in range(8):
                                    pss = [psF1.tile([128, NQ], F32, tag="f1ps",
                                                     name=f"f1ps{i}")
                                           for i in range(4)]
                                    for cp in range(4):
                                        for fo4 in range(4):
                                            nc.tensor.matmul(
                                                pss[fo4][:],
                                                w1_t[fog * 4 + cp][
                                                    :, :, fo4 * 128 : (fo4 + 1) * 128],
                                                h2_8[cp][:],
                                                start=(cp == 0), stop=(cp == 3),
                                                perf_mode=mybir.MatmulPerfMode.DoubleRow)
                                    for fo4 in range(4):
                                        fo = fog * 4 + fo4
                                        nc.scalar.activation(
                                            g_t[fo][:], pss[fo4][:],
                                            GELU_AF or AF.Gelu,
                                            bias=fc1b_t[:, fo : fo + 1],
                                            scale=1.0 / W8SCALE)
                            # fc2 + residual
                            with ExitStack() as _stf:
                                w2_pool = _stf.enter_context(tc.tile_pool(name="w2p", bufs=24))
                                psF2 = _stf.enter_context(tc.tile_pool(name="psF2", bufs=8, space="PSUM"))
                                out_pool = _stf.enter_context(tc.tile_pool(name="op", bufs=4))
                                for cog in range(2):
                                    pss = [psF2.tile([128, NQ], F32, tag="f2ps",
                                                     name=f"f2ps{i}")
                                           for i in range(4)]
                                    for ko in range(FT):
                                        w2_t = w2_pool.tile([128, 512], BF16, tag="w2")
                                        _dma(nc, w2_t[:],
                                             fc2_w[ko * 128 : (ko + 1) * 128,
                                                   cog * 512 : (cog + 1) * 512])
                                        for co4 in range(4):
                                            nc.tensor.matmul(
                                                pss[co4][:],
                                                w2_t[:, co4 * 128 : (co4 + 1) * 128],
                                                g_t[ko][:],
                                                start=(ko == 0), stop=(ko == FT - 1))
                                    for co4 in range(4):
                                        co = cog * 4 + co4
                                        o_t = out_pool.tile([128, NQ], F32, tag="o")
                                        nc.vector.scalar_tensor_tensor(
                                            o_t[:], pss[co4][:], fc2b_t[:, co : co + 1],
                                            x2[co][:], op0=ALU.add, op1=ALU.add)
                                        _dma(nc, outT[co * 128 : (co + 1) * 128, :], o_t[:])


_CACHED_NC = None


def _get_nc():
    global _CACHED_NC
    if _CACHED_NC is None:
        _CACHED_NC = build_program()
    return _CACHED_NC


def make_in_maps(inputs):
    from ml_dtypes import bfloat16

    ins = {k: np.ascontiguousarray(np.asarray(v), dtype=np.float32)
           for k, v in inputs.items()}
    proj_b_eff = (ins["proj_b"]
                  + ins["qkv_b"][2048:].astype(np.float64)
                  @ ins["proj_w"].astype(np.float64)).astype(np.float32)
    from ml_dtypes import float8_e4m3
    fc1_wb = np.ascontiguousarray(
        (ins["fc1_w"].reshape(4, 2, 128, MLP).transpose(0, 2, 1, 3)
         * W8SCALE).reshape(DIM // 2, 2, MLP)).astype(float8_e4m3)
    fc2_wb = np.ascontiguousarray(ins["fc2_w"]).astype(bfloat16)
    g_rows = np.concatenate([ins["ln1_g"], ins["ln2_g"]]).astype(bfloat16)
    in_maps = []
    for core in range(N_CORES):
        b = core // 4
        r = core % 4
        hsl = slice(r * GW, (r + 1) * GW)
        cols = [ins["ln1_g"], ins["ln1_b"], ins["ln2_g"], ins["ln2_b"],
                proj_b_eff, ins["fc2_b"], ins["fc1_b"],
                ins["qkv_b"][hsl], ins["qkv_b"][1024 + r * GW : 1024 + (r + 1) * GW]]
        packed = np.concatenate(
            [c.reshape(-1, 128).T for c in cols]
            + [np.ones((128, 8), np.float32),
               np.full((128, 1), 1.0 / W8SCALE, np.float32),
               np.ones((128, 3), np.float32)],
            axis=1)
        in_maps.append({
            "xT": np.ascontiguousarray(ins["x"][b].T).astype(bfloat16),
            "xresT": np.ascontiguousarray(
                ins["x"][b][r * NQ : (r + 1) * NQ, :].T).astype(bfloat16),
            "wqT": np.ascontiguousarray(ins["qkv_w"][:, hsl]).astype(bfloat16),
            "wkT": np.ascontiguousarray(
                ins["qkv_w"][:, 1024 + r * GW : 1024 + (r + 1) * GW]).astype(bfloat16),
            "wvT": np.ascontiguousarray(
                ins["qkv_w"][:, 2048 + r * GW : 2048 + (r + 1) * GW]).astype(bfloat16),
            "proj_wT": np.ascontiguousarray(ins["proj_w"][hsl, :]).astype(bfloat16),
            "fc1_w": fc1_wb,
            "fc2_w": fc2_wb,
            "ones_in": np.ones(128, bfloat16),
            "bias_pack": np.ascontiguousarray(packed),
            "g_rows": g_rows,
        })
    return in_maps


def gather_output(results):
    out = np.empty((2, NTOK, DIM), dtype=np.float32)
    for core in range(N_CORES):
        b = core // 4
        qs = (core % 4) * NQ
        out[b, qs : qs + NQ, :] = results[core]["outT"].T
    return out


def kernel(**inputs) -> np.ndarray:
    nc = _get_nc()
    in_maps = make_in_maps(inputs)
    res = run_bass_kernel_spmd(nc, in_maps, list(range(N_CORES)))
    return gather_output(res.results)


if __name__ == "__main__":
    rng = np.random.default_rng(0)
    demo = {
        "x": rng.standard_normal((2, NTOK, DIM), dtype=np.float32),
        "ln1_g": np.ones(DIM, np.float32), "ln1_b": np.zeros(DIM, np.float32),
        "qkv_w": (rng.standard_normal((DIM, 3 * DIM)) * 0.02).astype(np.float32),
        "qkv_b": np.zeros(3 * DIM, np.float32),
        "proj_w": (rng.standard_normal((DIM, DIM)) * 0.02).astype(np.float32),
        "proj_b": np.zeros(DIM, np.float32),
        "ln2_g": np.ones(DIM, np.float32), "ln2_b": np.zeros(DIM, np.float32),
        "fc1_w": (rng.standard_normal((DIM, MLP)) * 0.02).astype(np.float32),
        "fc1_b": np.zeros(MLP, np.float32),
        "fc2_w": (rng.standard_normal((MLP, DIM)) * 0.02).astype(np.float32),
        "fc2_b": np.zeros(DIM, np.float32),
    }
    out = kernel(**demo)
    print("out", out.shape, out.dtype, float(np.abs(out).max()))
